# revision 7
# baseline (speedup 1.0000x reference)
"""LowRankAttention Trainium2 kernel (Bass/Tile, 8-core data-parallel).

Math refactor vs the reference: the reference materializes
    kv = (x @ W.T)                      # (bs, ns, H*dm)
    scores = einsum('hrd,bhnd->bhrn', routers, kv)
but scores == x @ (routers_h @ W_h).T, so we precompute
    Q_all[hr, s] = (routers[h] @ W[h*dm:(h+1)*dm, :])[r, s]    # (512, 1024)
and never materialize kv:
    S_T[hr, n]   = Q_all @ x[b].T                  (PE, fp32r)
    A_T          = softmax(S_T * qk_scale, axis=n) -> attn output directly
    C_T[dc, hr]  = x[b].T @ A_all.T   (all heads in one matmul group)
    out5[h]      = C_T[:, h].T @ W_h.T             (free dim = 256)
    out[b]       = out_all @ proj.T + bias         (deferred, proj.T streamed)
All matmuls run fp32r with output free-dim >= 256 (1 cycle/row).
"""

import numpy as np

BPC = 4          # batch elems per core (32 / 8 cores)
NS = 512         # ns
SEQ = 1024       # seq_len == d_model-space of W rows
H = 8
DR = 64
DM = 256
HD = H * DM      # 2048
HR = H * DR      # 512
P = 128
QK = 1.0 / 16.0  # 1/sqrt(256)
NCORES = 8

_cache = {}


def _build():
    from contextlib import ExitStack

    import concourse.bacc as bacc
    import concourse.mybir as mybir
    from concourse import tile
    from concourse.masks import make_identity

    f32 = mybir.dt.float32
    f32r = mybir.dt.float32r
    AX = mybir.AxisListType.X
    EXP = mybir.ActivationFunctionType.Exp
    COPY = mybir.ActivationFunctionType.Copy

    def r(ap):
        return ap.bitcast(f32r)

    nc = bacc.Bacc("TRN2", target_bir_lowering=False, debug=False)

    x_d = nc.dram_tensor("x", (BPC, NS, SEQ), f32, kind="ExternalInput").ap()
    r_d = nc.dram_tensor("routers", (H, DR, DM), f32, kind="ExternalInput").ap()
    w_d = nc.dram_tensor("w_kv", (HD, SEQ), f32, kind="ExternalInput").ap()
    p_d = nc.dram_tensor("proj", (SEQ, HD), f32, kind="ExternalInput").ap()
    b_d = nc.dram_tensor("bias", (SEQ,), f32, kind="ExternalInput").ap()
    out_d = nc.dram_tensor("out", (BPC, DR, SEQ), f32, kind="ExternalOutput").ap()
    attn_d = nc.dram_tensor("attn", (BPC, H, DR, NS), f32, kind="ExternalOutput").ap()
    ptT_d = nc.dram_tensor("projT_tmp", (HD, SEQ), f32, kind="Internal").ap()

    with tile.TileContext(nc) as tc, ExitStack() as root:
        const = root.enter_context(tc.tile_pool(name="const", bufs=1))
        ident32 = const.tile([P, P], f32)
        make_identity(nc, ident32)
        ident = const.tile([P, P], f32)
        nc.scalar.copy(r(ident), ident32)
        biasrep = const.tile([DR, SEQ], f32)
        nc.sync.dma_start(
            out=biasrep, in_=b_d.rearrange("(o s) -> o s", o=1).broadcast_to([DR, SEQ])
        )

        # transpose helper: src (p,q) sbuf block -> psum block (q,p)
        def peT(psum_blk, src_blk, rnd=True):
            # out = src.T, so out free == src partition count == identity k == n
            pp = src_blk.shape[0]
            if rnd:
                nc.tensor.transpose(r(psum_blk), r(src_blk), r(ident[:pp, :pp]))
            else:
                nc.tensor.transpose(psum_blk, src_blk, ident32[:pp, :pp])

        pWT = root.enter_context(tc.tile_pool(name="pWT", bufs=1))
        pQT = root.enter_context(tc.tile_pool(name="pQT", bufs=1))
        pOT = root.enter_context(tc.tile_pool(name="pOT", bufs=1))
        psm = root.enter_context(tc.tile_pool(name="psm", bufs=1))
        pofin = root.enter_context(tc.tile_pool(name="pofin", bufs=2))

        # long-lived sbuf tensors
        wt = [pWT.tile([P, HD], f32, name=f"wt{j}") for j in range(8)]       # W.T
        qt = [pQT.tile([P, HR], f32, name=f"qt{j}") for j in range(8)]       # Q_all.T
        otall = [pOT.tile([P, BPC * DR], f32, name=f"ot{j}") for j in range(16)]

        with tc.tile_pool(name="wps", bufs=6, space="PSUM") as wps:

            def ps(pp=P, ff=512, name="ps"):
                return wps.tile([pp, ff], f32, name=name, tag="ps")

            # ---------------- prep: W.T, Q_all.T, routersT, proj.T->DRAM ----
            with ExitStack() as prep:
                pwn = prep.enter_context(tc.tile_pool(name="pwn", bufs=4))
                prt = prep.enter_context(tc.tile_pool(name="prt", bufs=1))
                pqtmp = prep.enter_context(tc.tile_pool(name="pqtmp", bufs=3))
                ppn = prep.enter_context(tc.tile_pool(name="ppn", bufs=2))
                ppst = prep.enter_context(tc.tile_pool(name="ppst", bufs=3))

                # routers (512,256) -> routT[kk] (128, 512)  kk in 0..2
                routT = [prt.tile([P, HR], f32, name=f"routT{kk}") for kk in range(2)]
                for t in range(4):
                    rnat = prt.tile([P, DM], f32, name="rnat", tag="rnat", bufs=2)
                    nc.sync.dma_start(
                        out=r(rnat),
                        in_=r(r_d.rearrange("h r d -> (h r) d")[t * P:(t + 1) * P, :]),
                    )
                    for kk in range(2):
                        pb = ps(name="psR")
                        peT(pb[:, :P], rnat[:, kk * P:(kk + 1) * P])
                        nc.scalar.copy(r(routT[kk][:, t * P:(t + 1) * P]), pb[:, :P])

                wnat = []
                for i in range(16):
                    wn = pwn.tile([P, SEQ], f32, name="wn", tag="wn")
                    nc.sync.dma_start(out=r(wn), in_=r(w_d[i * P:(i + 1) * P, :]))
                    wnat.append(wn)
                    # W.T blocks: wt[j][:, i*128:+128] = T(wn[:, j*128:+128])
                    for j in range(8):
                        pb = ps(name="psW")
                        peT(pb[:, :P], wn[:, j * P:(j + 1) * P])
                        nc.scalar.copy(r(wt[j][:, i * P:(i + 1) * P]), pb[:, :P])
                    if i % 2 == 1:
                        # Q for head h: Q[h] = routers[h] @ W_h  (64, 1024)
                        h = i // 2
                        qtmp = pqtmp.tile([DR, SEQ], f32, name="qtmp", tag="qtmp")
                        for half in range(2):
                            qp = ps(DR, 512, name="psQ")
                            for kk in range(2):
                                nc.tensor.matmul(
                                    qp,
                                    r(routT[kk][:, h * DR:(h + 1) * DR]),
                                    r(wnat[2 * h + kk][:, half * 512:(half + 1) * 512]),
                                    start=(kk == 0),
                                    stop=(kk == 1),
                                )
                            nc.vector.tensor_copy(
                                r(qtmp[:, half * 512:(half + 1) * 512]), qp
                            )
                        # transpose Q[h] -> qt[j][:, h*64:+64]
                        for j in range(8):
                            qb = ps(name="psQT")
                            peT(qb[:, :DR], qtmp[:, j * P:(j + 1) * P])
                            nc.scalar.copy(r(qt[j][:, h * DR:(h + 1) * DR]), qb[:, :DR])
                    if i == 1:
                        wnat_pair_done = True

                # proj (1024, 2048) -> projT (2048, 1024) staged via DRAM
                for i in range(8):
                    pn = ppn.tile([P, HD], f32, name="pn", tag="pn")
                    nc.sync.dma_start(out=r(pn), in_=r(p_d[i * P:(i + 1) * P, :]))
                    for g in range(4):
                        pb = ps(name="psP")
                        for jj in range(4):
                            peT(
                                pb[:, jj * P:(jj + 1) * P],
                                pn[:, (g * 4 + jj) * P:(g * 4 + jj + 1) * P],
                            )
                        st = ppst.tile([P, 512], f32, name="pst", tag="pst")
                        nc.scalar.copy(r(st), pb)
                        # scatter the 4 transposed blocks to 4 row-chunks of projT
                        # (both APs iterate p-major, then j, then q)
                        nc.sync.dma_start(
                            out=r(ptT_d.rearrange("(j p) s -> p j s", p=P)[
                                :, g * 4:(g + 1) * 4, i * P:(i + 1) * P
                            ]),
                            in_=r(st.rearrange("p (j q) -> p j q", q=P)),
                        )

            # ---------------- phase A: per-batch pipeline --------------------
            with ExitStack() as pha:
                pxn = pha.enter_context(tc.tile_pool(name="pxn", bufs=6))
                pxt = pha.enter_context(tc.tile_pool(name="pxt", bufs=10))
                pan = pha.enter_context(tc.tile_pool(name="pan", bufs=4))
                pat = pha.enter_context(tc.tile_pool(name="pat", bufs=4))
                pct = pha.enter_context(tc.tile_pool(name="pct", bufs=8))
                pt5 = pha.enter_context(tc.tile_pool(name="pt5", bufs=3))
                psml = pha.enter_context(tc.tile_pool(name="psml", bufs=4))

                for b in range(BPC):
                    xnat = []
                    for t in range(4):
                        xn = pxn.tile([P, SEQ], f32, name="xn", tag="xn")
                        nc.sync.dma_start(out=r(xn), in_=r(x_d[b, t * P:(t + 1) * P, :]))
                        xnat.append(xn)

                    # x[b].T : xt[j] (128, 512)
                    xt = []
                    for j in range(8):
                        pb = ps(name="psXT")
                        for t in range(4):
                            peT(pb[:, t * P:(t + 1) * P], xnat[t][:, j * P:(j + 1) * P])
                        xtj = pxt.tile([P, NS], f32, name="xt", tag="xt")
                        nc.vector.tensor_copy(r(xtj), pb)
                        xt.append(xtj)

                    # S_T (hr, n) + softmax -> anat (A_T), DMA out, transpose -> at
                    anat = []
                    for mt in range(4):
                        sp = ps(name="psS")
                        for j in range(8):
                            nc.tensor.matmul(
                                sp,
                                r(qt[j][:, mt * P:(mt + 1) * P]),
                                r(xt[j]),
                                start=(j == 0),
                                stop=(j == 7),
                            )
                        nmax = psml.tile([P, 1], f32, name="nmax", tag="nmax")
                        nc.vector.reduce_max(nmax, sp, axis=AX, negate=True)
                        nms = psml.tile([P, 1], f32, name="nms", tag="nms")
                        nc.vector.tensor_scalar_mul(nms, nmax, QK)
                        am = pan.tile([P, NS], f32, name="am", tag="am")
                        ssum = psml.tile([P, 1], f32, name="ssum", tag="ssum")
                        nc.scalar.activation(
                            am, sp, EXP, bias=nms, scale=QK, accum_out=ssum
                        )
                        rsum = psml.tile([P, 1], f32, name="rsum", tag="rsum")
                        nc.vector.reciprocal(rsum, ssum)
                        nc.scalar.activation(am, am, COPY, scale=rsum)
                        nc.sync.dma_start(
                            out=attn_d[b].rearrange("h r n -> (h r) n")[
                                mt * P:(mt + 1) * P, :
                            ],
                            in_=am,
                        )
                        anat.append(am)

                    at = []
                    for t in range(4):
                        pb = ps(name="psAT")
                        for mt in range(4):
                            peT(pb[:, mt * P:(mt + 1) * P], anat[mt][:, t * P:(t + 1) * P], rnd=False)
                        att = pat.tile([P, HR], f32, name="at", tag="at")
                        nc.scalar.copy(r(att), pb)
                        at.append(att)

                    # C_T (dcol, hr) all heads at once
                    ct = []
                    for dj in range(8):
                        cp = ps(name="psC")
                        for t in range(4):
                            nc.tensor.matmul(
                                cp,
                                r(xnat[t][:, dj * P:(dj + 1) * P]),
                                r(at[t]),
                                start=(t == 0),
                                stop=(t == 3),
                            )
                        ctj = pct.tile([P, HR], f32, name="ct", tag="ct")
                        nc.vector.tensor_copy(r(ctj), cp)
                        ct.append(ctj)

                    # out5[h] = C_T[:,h].T @ W_h.T  (64, 256); transpose into otall
                    for h in range(8):
                        op = ps(DR, DM, name="ps5")
                        for dj in range(8):
                            nc.tensor.matmul(
                                op,
                                r(ct[dj][:, h * DR:(h + 1) * DR]),
                                r(wt[dj][:, h * DM:(h + 1) * DM]),
                                start=(dj == 0),
                                stop=(dj == 7),
                            )
                        t5 = pt5.tile([DR, DM], f32, name="t5", tag="t5")
                        nc.vector.tensor_copy(r(t5), op)
                        tp = ps(P, P, name="psO")
                        peT(tp[:, :DR], t5[:, :P])
                        peT(tp[:, DR:2 * DR], t5[:, P:2 * P])
                        nc.scalar.copy(
                            r(otall[2 * h][:, b * DR:(b + 1) * DR]), tp[:, :DR]
                        )
                        nc.scalar.copy(
                            r(otall[2 * h + 1][:, b * DR:(b + 1) * DR]), tp[:, DR:2 * DR]
                        )

        # ---------------- phase B: out = out_all @ proj.T + bias ------------
        with tc.tile_pool(name="aps", bufs=1, space="PSUM") as aps, \
             tc.tile_pool(name="pptl", bufs=3) as pptl:
            acc = [
                aps.tile([DR, 512], f32, name=f"acc{b}_{sc}", tag=f"acc{b}{sc}")
                for b in range(BPC) for sc in range(2)
            ]
            for j in range(16):
                ptl = pptl.tile([P, SEQ], f32, name="ptl", tag="ptl")
                nc.sync.dma_start(out=r(ptl), in_=r(ptT_d[j * P:(j + 1) * P, :]))
                for b in range(BPC):
                    for sc in range(2):
                        nc.tensor.matmul(
                            acc[b * 2 + sc],
                            r(otall[j][:, b * DR:(b + 1) * DR]),
                            r(ptl[:, sc * 512:(sc + 1) * 512]),
                            start=(j == 0),
                            stop=(j == 15),
                        )
            for b in range(BPC):
                for sc in range(2):
                    of = pofin.tile([DR, 512], f32, name="of", tag="of")
                    nc.vector.tensor_tensor(
                        out=of,
                        in0=acc[b * 2 + sc],
                        in1=biasrep[:, sc * 512:(sc + 1) * 512],
                        op=mybir.AluOpType.add,
                    )
                    nc.sync.dma_start(
                        out=out_d[b, :, sc * 512:(sc + 1) * 512], in_=of
                    )

    nc.compile()
    return nc


def _get_nc():
    if "nc" not in _cache:
        _cache["nc"] = _build()
    return _cache["nc"]


def kernel(x, routers, w_kv_weight, proj_weight, proj_bias):
    from concourse import bass_utils

    nc = _get_nc()
    f = np.float32
    shared = {
        "routers": np.ascontiguousarray(routers, dtype=f),
        "w_kv": np.ascontiguousarray(w_kv_weight, dtype=f),
        "proj": np.ascontiguousarray(proj_weight, dtype=f),
        "bias": np.ascontiguousarray(proj_bias, dtype=f),
    }
    in_maps = [
        {"x": np.ascontiguousarray(x[c * BPC:(c + 1) * BPC], dtype=f), **shared}
        for c in range(NCORES)
    ]
    res = bass_utils.run_bass_kernel_spmd(nc, in_maps, list(range(NCORES)))
    out = np.concatenate([res.results[c]["out"] for c in range(NCORES)], axis=0)
    attn = np.concatenate([res.results[c]["attn"] for c in range(NCORES)], axis=0)
    return out, attn
